# revision 6
# baseline (speedup 1.0000x reference)
"""2-layer GCN (segment-mean -> Linear -> ReLU -> segment-mean -> Linear) on
8 Trainium2 NeuronCores.

Strategy:
- Shard dst nodes (and their incident edges) across the 8 cores.
- Edges sorted by (dst tile, src bucket); src features fetched with
  dma_gather (4 parallel SWDGE queues) as bf16 rows.
- Segment-sum per 128-dst tile via one-hot matmuls accumulated in PSUM;
  one-hot matrices built in a single DVE is_equal op per tile.
- Linear layers fused per tile (W1 + ReLU + W2); the [N, 64] intermediate
  z = relu(...) @ W2 is exchanged with 4 quarter-AllGathers issued as layer 1
  progresses, so the exchange overlaps the tail of layer 1's gathers and
  layer 2 starts without a full barrier.
- Uses the algebraic identity segmean(h) @ W = segmean(h @ W).
"""
import os
import numpy as np
import ml_dtypes

import concourse.bass as bass
import concourse.mybir as mybir
from concourse import bacc
from concourse.tile import TileContext
from concourse.tile_rust import add_dep_helper
from concourse.masks import make_identity
from concourse.bass_utils import run_bass_kernel_spmd

P = 128
NCORES = 8
SRC_CHUNK = 25600           # int16-addressable table chunk (layer 1)
BF16 = ml_dtypes.bfloat16

LAST_EXEC_NS = None


def _bucketize(s_all, d_all, n_tiles, n_buckets, bucket_of, locidx_of, shard):
    """Build the uniform slot structure for one layer's gathers."""
    per_core = []
    counts = np.zeros((NCORES, n_tiles, n_buckets), np.int64)
    for c in range(NCORES):
        s, d = s_all[c], d_all[c]
        tile = d // P
        bucket = bucket_of(s)
        key = tile * n_buckets + bucket
        order = np.argsort(key, kind="stable")
        s, d, key = s[order], d[order], key[order]
        counts[c] = np.bincount(key, minlength=n_tiles * n_buckets).reshape(n_tiles, n_buckets)
        per_core.append((s, d, key))

    Lt_true = counts.max(axis=0)
    L_tb = ((Lt_true + P - 1) // P) * P
    empty = L_tb.sum(axis=1) == 0
    L_tb[empty, 0] = P
    Lt_true = np.maximum(Lt_true, (L_tb > 0).astype(np.int64))
    nch_tb = L_tb // P
    nch_t = nch_tb.sum(axis=1)
    total_chunks = int(nch_t.sum())
    total_slots = total_chunks * P
    chunk_off_tb = np.concatenate([[0], np.cumsum(nch_tb.reshape(-1))])[:-1].reshape(n_tiles, n_buckets)

    idx_list, dl_list = [], []
    for c in range(NCORES):
        s, d, key = per_core[c]
        idx16 = np.zeros(total_slots, np.int16)
        dloc = np.full(total_slots, 999.0, np.float32)
        cnt_flat = counts[c].reshape(-1)
        seg_start = np.concatenate([[0], np.cumsum(cnt_flat)])[:-1]
        slots = (chunk_off_tb.reshape(-1) * P)[key] + (np.arange(len(s)) - seg_start[key])
        idx16[slots] = locidx_of(s).astype(np.int16)
        dloc[slots] = (d % P).astype(np.float32)
        w = idx16.reshape(-1, 16).T
        idx_list.append(np.tile(w, (8, 1)))                 # [128, S/16]
        dl_list.append(dloc.reshape(total_chunks, P).T.astype(BF16))

    return dict(L_tb=L_tb, Lt_true=Lt_true, nch_t=nch_t, chunk_off_tb=chunk_off_tb,
                total_chunks=total_chunks, total_slots=total_slots,
                idx=idx_list, dl=dl_list)


def _prep(src, dst, n_nodes):
    shard = n_nodes // NCORES
    n_tiles = (shard + P - 1) // P
    n_buckets = (n_nodes + SRC_CHUNK - 1) // SRC_CHUNK
    Q = shard // 4                                 # quarter size (3125)

    src = np.asarray(src, np.int64)
    dst = np.asarray(dst, np.int64)
    s_all, d_all, cnt_nodes = [], [], []
    for c in range(NCORES):
        m = (dst >= c * shard) & (dst < (c + 1) * shard)
        s_all.append(src[m])
        d = dst[m] - c * shard
        d_all.append(d)
        cnt_nodes.append(np.bincount(d, minlength=shard).astype(np.float64))

    st1 = _bucketize(s_all, d_all, n_tiles, n_buckets,
                     lambda s: s // SRC_CHUNK,
                     lambda s: s - (s // SRC_CHUNK) * SRC_CHUNK, shard)
    # layer 2: bucket k holds global nodes {c*shard + [k*Q, (k+1)*Q)}; after
    # quarter-AllGather k they live contiguously in z_q[k] at row c*Q + r.
    st2 = _bucketize(s_all, d_all, n_tiles, 4,
                     lambda s: (s % shard) // Q,
                     lambda s: (s // shard) * Q + (s % shard) % Q, shard)

    data = []
    for c in range(NCORES):
        cnt_node = cnt_nodes[c]
        recip = (1.0 / np.maximum(cnt_node, 1.0)).astype(np.float32)
        cntp = np.maximum(cnt_node, 1.0).astype(np.float32)
        recip_pad = np.ones(n_tiles * P, np.float32)
        recip_pad[:shard] = recip
        cntp_pad = np.ones(n_tiles * P, np.float32)
        cntp_pad[:shard] = cntp
        rb = np.broadcast_to(recip_pad.reshape(n_tiles, 1, P), (n_tiles, P, P)).astype(BF16)
        data.append(dict(idx=st1["idx"][c], dl=st1["dl"][c],
                         idx2=st2["idx"][c], dl2=st2["dl"][c],
                         rb=np.ascontiguousarray(rb),
                         rc=recip_pad.reshape(n_tiles, P, 1).astype(np.float32),
                         cr=cntp_pad.reshape(n_tiles, 1, P).astype(BF16)))

    struct = dict(n_tiles=n_tiles, n_buckets=n_buckets, shard=shard, Q=Q,
                  st1=st1, st2=st2)
    return struct, data


def _build(st, n_nodes):
    n_tiles, n_buckets = st["n_tiles"], st["n_buckets"]
    shard, Q = st["shard"], st["Q"]
    st1, st2 = st["st1"], st["st2"]
    max_nch = int(max(st1["nch_t"].max(), st2["nch_t"].max()))
    f32, bf16, i16 = mybir.dt.float32, mybir.dt.bfloat16, mybir.dt.int16
    # tile index after which quarter k of z_local is fully written
    q_tile = [((k + 1) * Q - 1) // P for k in range(4)]

    nc = bacc.Bacc("TRN2", target_bir_lowering=False, debug=False,
                   num_devices=NCORES, num_swdge_queues=4,
                   dynamic_dma_scratch_size=32768)
    X_d = nc.dram_tensor("X", [n_nodes, P], bf16, kind="ExternalInput")
    W1_d = nc.dram_tensor("W1b", [P, P], bf16, kind="ExternalInput")
    W2_d = nc.dram_tensor("W2b", [P, 64], bf16, kind="ExternalInput")
    b1_d = nc.dram_tensor("b1c", [P, 1], f32, kind="ExternalInput")
    b2_d = nc.dram_tensor("b2r", [1, 64], bf16, kind="ExternalInput")
    iota_d = nc.dram_tensor("iota", [P, P], bf16, kind="ExternalInput")
    idx_d = nc.dram_tensor("idx", [P, st1["total_slots"] // 16], i16, kind="ExternalInput")
    dl_d = nc.dram_tensor("dl", [P, st1["total_chunks"]], bf16, kind="ExternalInput")
    idx2_d = nc.dram_tensor("idx2", [P, st2["total_slots"] // 16], i16, kind="ExternalInput")
    dl2_d = nc.dram_tensor("dl2", [P, st2["total_chunks"]], bf16, kind="ExternalInput")
    rb_d = nc.dram_tensor("rb", [n_tiles, P, P], bf16, kind="ExternalInput")
    rc_d = nc.dram_tensor("rc", [n_tiles, P, 1], f32, kind="ExternalInput")
    cr_d = nc.dram_tensor("cr", [n_tiles, 1, P], bf16, kind="ExternalInput")
    out_d = nc.dram_tensor("out", [shard, 64], f32, kind="ExternalOutput")

    z_local = nc.dram_tensor("z_local", [shard, P], bf16)
    z_q = [nc.dram_tensor(f"z_q{k}", [NCORES * Q, P], bf16, addr_space="Shared")
           for k in range(4)]

    qn = [0]
    cc_insts = {}

    with TileContext(nc) as tc:
        with tc.tile_pool(name="const", bufs=1) as cpool, \
             tc.tile_pool(name="g", bufs=6) as gpool, \
             tc.tile_pool(name="oh", bufs=6) as ohpool, \
             tc.tile_pool(name="wk", bufs=3) as wpool, \
             tc.tile_pool(name="sm", bufs=3) as smpool, \
             tc.tile_pool(name="ps1", bufs=2, space="PSUM") as ps1, \
             tc.tile_pool(name="ps2", bufs=2, space="PSUM") as ps2, \
             tc.tile_pool(name="ps3", bufs=2, space="PSUM") as ps3, \
             tc.tile_pool(name="ps4", bufs=2, space="PSUM") as ps4:

            W1sb = cpool.tile([P, P], bf16)
            nc.sync.dma_start(out=W1sb[:], in_=W1_d[:])
            W2sb = cpool.tile([P, 64], bf16)
            nc.sync.dma_start(out=W2sb[:], in_=W2_d[:])
            b1sb = cpool.tile([P, 1], f32)
            nc.sync.dma_start(out=b1sb[:], in_=b1_d[:])
            b2sb = cpool.tile([1, 64], bf16)
            nc.sync.dma_start(out=b2sb[:], in_=b2_d[:])
            iotasb = cpool.tile([P, P], bf16)
            nc.sync.dma_start(out=iotasb[:], in_=iota_d[:])
            idxsb = cpool.tile([P, st1["total_slots"] // 16], i16)
            nc.sync.dma_start(out=idxsb[:], in_=idx_d[:])
            dlsb = cpool.tile([P, st1["total_chunks"]], bf16)
            nc.sync.dma_start(out=dlsb[:], in_=dl_d[:])
            idx2sb = cpool.tile([P, st2["total_slots"] // 16], i16)
            nc.sync.dma_start(out=idx2sb[:], in_=idx2_d[:])
            dl2sb = cpool.tile([P, st2["total_chunks"]], bf16)
            nc.sync.dma_start(out=dl2sb[:], in_=dl2_d[:])
            ident = cpool.tile([P, P], bf16)
            make_identity(nc, ident[:])

            for layer in (0, 1):
                s = st1 if layer == 0 else st2
                L_tb, Lt_true = s["L_tb"], s["Lt_true"]
                nch_t, chunk_off_tb = s["nch_t"], s["chunk_off_tb"]
                ixsb = idxsb if layer == 0 else idx2sb
                dsb = dlsb if layer == 0 else dl2sb
                nb = n_buckets if layer == 0 else 4
                for t in range(n_tiles):
                    nch = int(nch_t[t])
                    G = gpool.tile([P, max_nch * P], bf16, tag="G")
                    for b in range(nb):
                        L = int(L_tb[t, b])
                        if L == 0:
                            continue
                        if layer == 0:
                            table = X_d[b * SRC_CHUNK:min((b + 1) * SRC_CHUNK, n_nodes), :]
                        else:
                            table = z_q[b][:]
                        co = int(chunk_off_tb[t, b] - chunk_off_tb[t, 0])
                        gco = int(chunk_off_tb[t, b])
                        # first few tiles emit full padded counts so G pool
                        # buffers never expose uninitialized SBUF to the MMs
                        ntrue = L if (layer == 0 and t < 6) else int(Lt_true[t, b])
                        ncols = (ntrue + 15) // 16
                        gi = nc.gpsimd.dma_gather(
                            G[:, co * P:(co + L // P) * P].rearrange("p (c d) -> p c d", d=P),
                            table,
                            ixsb[:, gco * 8:gco * 8 + ncols],
                            ntrue, ntrue, P,
                            queue_num=qn[0] % 4,
                        )
                        qn[0] += 1
                        if layer == 1:
                            add_dep_helper(gi.ins, cc_insts[b], sync=True,
                                           reason="z quarter ready")
                    oh = ohpool.tile([P, max_nch * P], bf16, tag="oh")
                    dcol0 = int(chunk_off_tb[t, 0])
                    in0 = iotasb[:].rearrange("p (o d) -> p o d", o=1).broadcast_to([P, nch, P])
                    in1 = dsb[:, dcol0:dcol0 + nch].rearrange("p (c o) -> p c o", o=1).broadcast_to([P, nch, P])
                    nc.vector.tensor_tensor(
                        out=oh[:, :nch * P].rearrange("p (c d) -> p c d", d=P),
                        in0=in0, in1=in1, op=mybir.AluOpType.is_equal)
                    psum1 = ps1.tile([P, P], f32, space="PSUM", tag="p1")
                    for cci in range(nch):
                        nc.tensor.matmul(
                            out=psum1[:], lhsT=G[:, cci * P:(cci + 1) * P],
                            rhs=oh[:, cci * P:(cci + 1) * P],
                            start=(cci == 0), stop=(cci == nch - 1))
                    rows = min(P, shard - t * P)
                    if layer == 0:
                        rbt = smpool.tile([P, P], bf16, tag="rbt")
                        nc.sync.dma_start(out=rbt[:], in_=rb_d[t])
                        m1 = wpool.tile([P, P], bf16, tag="m1")
                        nc.vector.tensor_tensor(out=m1[:], in0=psum1[:], in1=rbt[:],
                                                op=mybir.AluOpType.mult)
                        psum2 = ps2.tile([P, P], f32, space="PSUM", tag="p2")
                        nc.tensor.matmul(out=psum2[:], lhsT=W1sb[:], rhs=m1[:],
                                         start=True, stop=True)
                        h1T = wpool.tile([P, P], bf16, tag="h1T")
                        nc.scalar.activation(out=h1T[:], in_=psum2[:],
                                             func=mybir.ActivationFunctionType.Relu,
                                             bias=b1sb[:, :1], scale=1.0)
                        psum3 = ps3.tile([64, P], f32, space="PSUM", tag="p3")
                        nc.tensor.matmul(out=psum3[:], lhsT=W2sb[:], rhs=h1T[:],
                                         start=True, stop=True)
                        zT = wpool.tile([64, P], bf16, tag="zT")
                        nc.scalar.activation(out=zT[:], in_=psum3[:],
                                             func=mybir.ActivationFunctionType.Copy,
                                             scale=1.0)
                        psum4 = ps4.tile([P, 64], f32, space="PSUM", tag="p4")
                        nc.tensor.matmul(out=psum4[:], lhsT=zT[:], rhs=ident[:64, :64],
                                         start=True, stop=True)
                        zt = wpool.tile([P, 64], bf16, tag="zt")
                        nc.scalar.activation(out=zt[:], in_=psum4[:],
                                             func=mybir.ActivationFunctionType.Copy,
                                             scale=1.0)
                        nc.sync.dma_start(out=z_local[t * P:t * P + rows, :64],
                                          in_=zt[:rows, :])
                        for k in range(4):
                            if q_tile[k] == t:
                                cc = nc.gpsimd.collective_compute(
                                    "AllGather", mybir.AluOpType.bypass,
                                    replica_groups=[list(range(NCORES))],
                                    ins=[z_local[k * Q:(k + 1) * Q, :]],
                                    outs=[z_q[k][:]])
                                cc_insts[k] = cc.ins
                    else:
                        s5 = wpool.tile([64, P], bf16, tag="zT")
                        nc.scalar.activation(out=s5[:], in_=psum1[:64, :],
                                             func=mybir.ActivationFunctionType.Copy,
                                             scale=1.0)
                        psum4b = ps4.tile([P, 64], f32, space="PSUM", tag="p4")
                        nc.tensor.matmul(out=psum4b[:], lhsT=s5[:], rhs=ident[:64, :64],
                                         start=True, stop=False)
                        crt = smpool.tile([1, P], bf16, tag="crt")
                        nc.sync.dma_start(out=crt[:], in_=cr_d[t])
                        nc.tensor.matmul(out=psum4b[:], lhsT=crt[:], rhs=b2sb[:],
                                         start=False, stop=True)
                        rct = smpool.tile([P, 1], f32, tag="rct")
                        nc.sync.dma_start(out=rct[:], in_=rc_d[t])
                        outt = wpool.tile([P, 64], f32, tag="outt")
                        nc.scalar.activation(out=outt[:], in_=psum4b[:],
                                             func=mybir.ActivationFunctionType.Copy,
                                             scale=rct[:, :1])
                        nc.sync.dma_start(out=out_d[t * P:t * P + rows, :],
                                          in_=outt[:rows, :])
    nc.compile()
    return nc


def _gcn(features, W1, b1, W2, b2, src, dst):
    global LAST_EXEC_NS
    n_nodes = features.shape[0]
    st, data = _prep(src, dst, n_nodes)

    X16 = np.ascontiguousarray(np.asarray(features, np.float32)).astype(BF16)
    iota_host = np.tile(np.arange(P, dtype=np.float32)[None, :], (P, 1)).astype(BF16)
    common = dict(
        X=X16,
        W1b=np.asarray(W1, np.float32).astype(BF16),
        W2b=np.asarray(W2, np.float32).astype(BF16),
        b1c=np.asarray(b1, np.float32).reshape(P, 1),
        b2r=np.asarray(b2, np.float32).reshape(1, 64).astype(BF16),
        iota=iota_host,
    )
    in_maps = []
    for c in range(NCORES):
        d = data[c]
        in_maps.append(dict(common, idx=d["idx"], dl=d["dl"], idx2=d["idx2"],
                            dl2=d["dl2"], rb=d["rb"], rc=d["rc"], cr=d["cr"]))

    nc = _build(st, n_nodes)
    trace = bool(os.environ.get("GCN_TRACE"))
    try:
        res = run_bass_kernel_spmd(nc, in_maps, list(range(NCORES)), trace=trace,
                                   tmpdir=os.environ.get("GCN_TMPDIR"))
    except Exception:
        if not trace:
            raise
        res = run_bass_kernel_spmd(nc, in_maps, list(range(NCORES)))
    LAST_EXEC_NS = res.exec_time_ns
    out = np.concatenate([res.results[c]["out"] for c in range(NCORES)], axis=0)
    return np.ascontiguousarray(out, dtype=np.float32)


def kernel(features, W1, b1, W2, b2, src, dst):
    return _gcn(features, W1, b1, W2, b2, src, dst)


# revision 9
# speedup vs baseline: 1.0435x; 1.0435x over previous
"""2-layer GCN (segment-mean -> Linear -> ReLU -> segment-mean -> Linear) on
8 Trainium2 NeuronCores.

Strategy:
- Shard dst nodes (and their incident edges) across the 8 cores.
- Edges sorted by (dst tile, src bucket); src features fetched with
  dma_gather (4 parallel SWDGE queues) as bf16 rows.
- Segment-sum per 128-dst tile via one-hot matmuls accumulated in PSUM;
  one-hot matrices built in a single DVE is_equal op per tile.
- Linear layers fused per tile (W1 + ReLU + W2); the [N, 64] intermediate
  z = relu(...) @ W2 is exchanged with 4 quarter-AllGathers issued as layer 1
  progresses, so the exchange overlaps the tail of layer 1's gathers and
  layer 2 starts without a full barrier.
- Uses the algebraic identity segmean(h) @ W = segmean(h @ W).
"""
import os
import numpy as np
import ml_dtypes

import concourse.bass as bass
import concourse.mybir as mybir
from concourse import bacc
from concourse.tile import TileContext
from concourse.tile_rust import add_dep_helper
from concourse.masks import make_identity
from concourse.bass_utils import run_bass_kernel_spmd

P = 128
NCORES = 8
SRC_CHUNK = 25600           # int16-addressable table chunk (layer 1)
BF16 = ml_dtypes.bfloat16

LAST_EXEC_NS = None


def _bucketize(s_all, d_all, n_tiles, n_buckets, bucket_of, locidx_of, shard):
    """Build the uniform slot structure for one layer's gathers."""
    per_core = []
    counts = np.zeros((NCORES, n_tiles, n_buckets), np.int64)
    for c in range(NCORES):
        s, d = s_all[c], d_all[c]
        tile = d // P
        bucket = bucket_of(s)
        key = tile * n_buckets + bucket
        order = np.argsort(key, kind="stable")
        s, d, key = s[order], d[order], key[order]
        counts[c] = np.bincount(key, minlength=n_tiles * n_buckets).reshape(n_tiles, n_buckets)
        per_core.append((s, d, key))

    Lt_true = counts.max(axis=0)
    L_tb = ((Lt_true + P - 1) // P) * P
    empty = L_tb.sum(axis=1) == 0
    L_tb[empty, 0] = P
    Lt_true = np.maximum(Lt_true, (L_tb > 0).astype(np.int64))
    nch_tb = L_tb // P
    nch_t = nch_tb.sum(axis=1)
    total_chunks = int(nch_t.sum())
    total_slots = total_chunks * P
    chunk_off_tb = np.concatenate([[0], np.cumsum(nch_tb.reshape(-1))])[:-1].reshape(n_tiles, n_buckets)

    idx_list, dl_list = [], []
    for c in range(NCORES):
        s, d, key = per_core[c]
        idx16 = np.zeros(total_slots, np.int16)
        dloc = np.full(total_slots, 999.0, np.float32)
        cnt_flat = counts[c].reshape(-1)
        seg_start = np.concatenate([[0], np.cumsum(cnt_flat)])[:-1]
        slots = (chunk_off_tb.reshape(-1) * P)[key] + (np.arange(len(s)) - seg_start[key])
        idx16[slots] = locidx_of(s).astype(np.int16)
        dloc[slots] = (d % P).astype(np.float32)
        w = idx16.reshape(-1, 16).T
        idx_list.append(np.tile(w, (8, 1)))                 # [128, S/16]
        dl_list.append(dloc.reshape(total_chunks, P).T.astype(BF16))

    return dict(L_tb=L_tb, Lt_true=Lt_true, nch_t=nch_t, chunk_off_tb=chunk_off_tb,
                total_chunks=total_chunks, total_slots=total_slots,
                idx=idx_list, dl=dl_list)


def _prep(src, dst, n_nodes):
    shard = n_nodes // NCORES
    n_tiles = (shard + P - 1) // P
    n_buckets = (n_nodes + SRC_CHUNK - 1) // SRC_CHUNK
    Q = shard // 4                                 # quarter size (3125)

    src = np.asarray(src, np.int64)
    dst = np.asarray(dst, np.int64)
    s_all, d_all, cnt_nodes = [], [], []
    for c in range(NCORES):
        m = (dst >= c * shard) & (dst < (c + 1) * shard)
        s_all.append(src[m])
        d = dst[m] - c * shard
        d_all.append(d)
        cnt_nodes.append(np.bincount(d, minlength=shard).astype(np.float64))

    st1 = _bucketize(s_all, d_all, n_tiles, n_buckets,
                     lambda s: s // SRC_CHUNK,
                     lambda s: s - (s // SRC_CHUNK) * SRC_CHUNK, shard)
    st2 = st1  # layer 2 reuses the same bucketing (gather from z_full)

    data = []
    for c in range(NCORES):
        cnt_node = cnt_nodes[c]
        recip = (1.0 / np.maximum(cnt_node, 1.0)).astype(np.float32)
        cntp = np.maximum(cnt_node, 1.0).astype(np.float32)
        recip_pad = np.ones(n_tiles * P, np.float32)
        recip_pad[:shard] = recip
        cntp_pad = np.ones(n_tiles * P, np.float32)
        cntp_pad[:shard] = cntp
        rb = np.broadcast_to(recip_pad.reshape(n_tiles, 1, P), (n_tiles, P, P)).astype(BF16)
        data.append(dict(idx=st1["idx"][c], dl=st1["dl"][c],
                         idx2=st2["idx"][c], dl2=st2["dl"][c],
                         rb=np.ascontiguousarray(rb),
                         rc=recip_pad.reshape(n_tiles, P, 1).astype(np.float32),
                         cr=cntp_pad.reshape(n_tiles, 1, P).astype(BF16)))

    struct = dict(n_tiles=n_tiles, n_buckets=n_buckets, shard=shard, Q=Q,
                  st1=st1, st2=st2)
    return struct, data


def _build(st, n_nodes):
    n_tiles, n_buckets = st["n_tiles"], st["n_buckets"]
    shard, Q = st["shard"], st["Q"]
    st1, st2 = st["st1"], st["st2"]
    max_nch = int(max(st1["nch_t"].max(), st2["nch_t"].max()))
    f32, bf16, i16 = mybir.dt.float32, mybir.dt.bfloat16, mybir.dt.int16

    nc = bacc.Bacc("TRN2", target_bir_lowering=False, debug=False,
                   num_devices=NCORES, num_swdge_queues=4,
                   dynamic_dma_scratch_size=32768)
    X_d = nc.dram_tensor("X", [n_nodes, P], bf16, kind="ExternalInput")
    W1_d = nc.dram_tensor("W1b", [P, P], bf16, kind="ExternalInput")
    W2_d = nc.dram_tensor("W2b", [P, 64], bf16, kind="ExternalInput")
    b1_d = nc.dram_tensor("b1c", [P, 1], f32, kind="ExternalInput")
    b2_d = nc.dram_tensor("b2r", [1, 64], bf16, kind="ExternalInput")
    iota_d = nc.dram_tensor("iota", [P, P], bf16, kind="ExternalInput")
    idx_d = nc.dram_tensor("idx", [P, st1["total_slots"] // 16], i16, kind="ExternalInput")
    dl_d = nc.dram_tensor("dl", [P, st1["total_chunks"]], bf16, kind="ExternalInput")
    idx2_d = nc.dram_tensor("idx2", [P, st2["total_slots"] // 16], i16, kind="ExternalInput")
    dl2_d = nc.dram_tensor("dl2", [P, st2["total_chunks"]], bf16, kind="ExternalInput")
    rb_d = nc.dram_tensor("rb", [n_tiles, P, P], bf16, kind="ExternalInput")
    rc_d = nc.dram_tensor("rc", [n_tiles, P, 1], f32, kind="ExternalInput")
    cr_d = nc.dram_tensor("cr", [n_tiles, 1, P], bf16, kind="ExternalInput")
    out_d = nc.dram_tensor("out", [shard, 64], f32, kind="ExternalOutput")

    z_local = nc.dram_tensor("z_local", [shard, P], bf16)
    z_full = nc.dram_tensor("z_full", [NCORES * shard, P], bf16, addr_space="Shared")

    qn = [0]

    with TileContext(nc) as tc:
        with tc.tile_pool(name="const", bufs=1) as cpool, \
             tc.tile_pool(name="g", bufs=6) as gpool, \
             tc.tile_pool(name="oh", bufs=6) as ohpool, \
             tc.tile_pool(name="wk", bufs=3) as wpool, \
             tc.tile_pool(name="sm", bufs=3) as smpool, \
             tc.tile_pool(name="ps1", bufs=2, space="PSUM") as ps1, \
             tc.tile_pool(name="ps2", bufs=2, space="PSUM") as ps2, \
             tc.tile_pool(name="ps3", bufs=2, space="PSUM") as ps3, \
             tc.tile_pool(name="ps4", bufs=2, space="PSUM") as ps4:

            W1sb = cpool.tile([P, P], bf16)
            nc.sync.dma_start(out=W1sb[:], in_=W1_d[:])
            W2sb = cpool.tile([P, 64], bf16)
            nc.sync.dma_start(out=W2sb[:], in_=W2_d[:])
            b1sb = cpool.tile([P, 1], f32)
            nc.sync.dma_start(out=b1sb[:], in_=b1_d[:])
            b2sb = cpool.tile([1, 64], bf16)
            nc.sync.dma_start(out=b2sb[:], in_=b2_d[:])
            iotasb = cpool.tile([P, P], bf16)
            nc.sync.dma_start(out=iotasb[:], in_=iota_d[:])
            idxsb = cpool.tile([P, st1["total_slots"] // 16], i16)
            nc.sync.dma_start(out=idxsb[:], in_=idx_d[:])
            dlsb = cpool.tile([P, st1["total_chunks"]], bf16)
            nc.sync.dma_start(out=dlsb[:], in_=dl_d[:])
            idx2sb = cpool.tile([P, st2["total_slots"] // 16], i16)
            nc.sync.dma_start(out=idx2sb[:], in_=idx2_d[:])
            dl2sb = cpool.tile([P, st2["total_chunks"]], bf16)
            nc.sync.dma_start(out=dl2sb[:], in_=dl2_d[:])
            ident = cpool.tile([P, P], bf16)
            make_identity(nc, ident[:])

            for layer in (0, 1):
                s = st1 if layer == 0 else st2
                L_tb, Lt_true = s["L_tb"], s["Lt_true"]
                nch_t, chunk_off_tb = s["nch_t"], s["chunk_off_tb"]
                ixsb = idxsb if layer == 0 else idx2sb
                dsb = dlsb if layer == 0 else dl2sb
                nb = n_buckets if layer == 0 else 4
                for t in range(n_tiles):
                    nch = int(nch_t[t])
                    G = gpool.tile([P, max_nch * P], bf16, tag="G")
                    for b in range(nb):
                        L = int(L_tb[t, b])
                        if L == 0:
                            continue
                        if layer == 0:
                            table = X_d[b * SRC_CHUNK:min((b + 1) * SRC_CHUNK, n_nodes), :]
                        else:
                            table = z_full[b * SRC_CHUNK:min((b + 1) * SRC_CHUNK, n_nodes), :]
                        co = int(chunk_off_tb[t, b] - chunk_off_tb[t, 0])
                        gco = int(chunk_off_tb[t, b])
                        # first few tiles emit full padded counts so G pool
                        # buffers never expose uninitialized SBUF to the MMs
                        ntrue = L if (layer == 0 and t < 6) else int(Lt_true[t, b])
                        ncols = (ntrue + 15) // 16
                        gi = nc.gpsimd.dma_gather(
                            G[:, co * P:(co + L // P) * P].rearrange("p (c d) -> p c d", d=P),
                            table,
                            ixsb[:, gco * 8:gco * 8 + ncols],
                            ntrue, ntrue, P,
                            queue_num=qn[0] % 4,
                        )
                        qn[0] += 1
                    oh = ohpool.tile([P, max_nch * P], bf16, tag="oh")
                    dcol0 = int(chunk_off_tb[t, 0])
                    in0 = iotasb[:].rearrange("p (o d) -> p o d", o=1).broadcast_to([P, nch, P])
                    in1 = dsb[:, dcol0:dcol0 + nch].rearrange("p (c o) -> p c o", o=1).broadcast_to([P, nch, P])
                    nc.vector.tensor_tensor(
                        out=oh[:, :nch * P].rearrange("p (c d) -> p c d", d=P),
                        in0=in0, in1=in1, op=mybir.AluOpType.is_equal)
                    psum1 = ps1.tile([P, P], f32, space="PSUM", tag="p1")
                    for cci in range(nch):
                        nc.tensor.matmul(
                            out=psum1[:], lhsT=G[:, cci * P:(cci + 1) * P],
                            rhs=oh[:, cci * P:(cci + 1) * P],
                            start=(cci == 0), stop=(cci == nch - 1))
                    rows = min(P, shard - t * P)
                    if layer == 0:
                        rbt = smpool.tile([P, P], bf16, tag="rbt")
                        nc.sync.dma_start(out=rbt[:], in_=rb_d[t])
                        m1 = wpool.tile([P, P], bf16, tag="m1")
                        nc.vector.tensor_tensor(out=m1[:], in0=psum1[:], in1=rbt[:],
                                                op=mybir.AluOpType.mult)
                        psum2 = ps2.tile([P, P], f32, space="PSUM", tag="p2")
                        nc.tensor.matmul(out=psum2[:], lhsT=W1sb[:], rhs=m1[:],
                                         start=True, stop=True)
                        h1T = wpool.tile([P, P], bf16, tag="h1T")
                        nc.scalar.activation(out=h1T[:], in_=psum2[:],
                                             func=mybir.ActivationFunctionType.Relu,
                                             bias=b1sb[:, :1], scale=1.0)
                        psum3 = ps3.tile([64, P], f32, space="PSUM", tag="p3")
                        nc.tensor.matmul(out=psum3[:], lhsT=W2sb[:], rhs=h1T[:],
                                         start=True, stop=True)
                        zT = wpool.tile([64, P], bf16, tag="zT")
                        nc.scalar.activation(out=zT[:], in_=psum3[:],
                                             func=mybir.ActivationFunctionType.Copy,
                                             scale=1.0)
                        psum4 = ps4.tile([P, 64], f32, space="PSUM", tag="p4")
                        nc.tensor.matmul(out=psum4[:], lhsT=zT[:], rhs=ident[:64, :64],
                                         start=True, stop=True)
                        zt = wpool.tile([P, 64], bf16, tag="zt")
                        nc.scalar.activation(out=zt[:], in_=psum4[:],
                                             func=mybir.ActivationFunctionType.Copy,
                                             scale=1.0)
                        nc.sync.dma_start(out=z_local[t * P:t * P + rows, :64],
                                          in_=zt[:rows, :])
                    else:
                        s5 = wpool.tile([64, P], bf16, tag="zT")
                        nc.scalar.activation(out=s5[:], in_=psum1[:64, :],
                                             func=mybir.ActivationFunctionType.Copy,
                                             scale=1.0)
                        psum4b = ps4.tile([P, 64], f32, space="PSUM", tag="p4")
                        nc.tensor.matmul(out=psum4b[:], lhsT=s5[:], rhs=ident[:64, :64],
                                         start=True, stop=False)
                        crt = smpool.tile([1, P], bf16, tag="crt")
                        nc.sync.dma_start(out=crt[:], in_=cr_d[t])
                        nc.tensor.matmul(out=psum4b[:], lhsT=crt[:], rhs=b2sb[:],
                                         start=False, stop=True)
                        rct = smpool.tile([P, 1], f32, tag="rct")
                        nc.sync.dma_start(out=rct[:], in_=rc_d[t])
                        outt = wpool.tile([P, 64], f32, tag="outt")
                        nc.scalar.activation(out=outt[:], in_=psum4b[:],
                                             func=mybir.ActivationFunctionType.Copy,
                                             scale=rct[:, :1])
                        nc.sync.dma_start(out=out_d[t * P:t * P + rows, :],
                                          in_=outt[:rows, :])
                if layer == 0:
                    nc.gpsimd.collective_compute(
                        "AllGather", mybir.AluOpType.bypass,
                        replica_groups=[list(range(NCORES))],
                        ins=[z_local[:]], outs=[z_full[:]])
    nc.compile()
    return nc


def _gcn(features, W1, b1, W2, b2, src, dst):
    global LAST_EXEC_NS
    n_nodes = features.shape[0]
    st, data = _prep(src, dst, n_nodes)

    X16 = np.ascontiguousarray(np.asarray(features, np.float32)).astype(BF16)
    iota_host = np.tile(np.arange(P, dtype=np.float32)[None, :], (P, 1)).astype(BF16)
    common = dict(
        X=X16,
        W1b=np.asarray(W1, np.float32).astype(BF16),
        W2b=np.asarray(W2, np.float32).astype(BF16),
        b1c=np.asarray(b1, np.float32).reshape(P, 1),
        b2r=np.asarray(b2, np.float32).reshape(1, 64).astype(BF16),
        iota=iota_host,
    )
    in_maps = []
    for c in range(NCORES):
        d = data[c]
        in_maps.append(dict(common, idx=d["idx"], dl=d["dl"], idx2=d["idx2"],
                            dl2=d["dl2"], rb=d["rb"], rc=d["rc"], cr=d["cr"]))

    nc = _build(st, n_nodes)
    trace = bool(os.environ.get("GCN_TRACE"))
    try:
        res = run_bass_kernel_spmd(nc, in_maps, list(range(NCORES)), trace=trace,
                                   tmpdir=os.environ.get("GCN_TMPDIR"))
    except Exception:
        if not trace:
            raise
        res = run_bass_kernel_spmd(nc, in_maps, list(range(NCORES)))
    LAST_EXEC_NS = res.exec_time_ns
    out = np.concatenate([res.results[c]["out"] for c in range(NCORES)], axis=0)
    return np.ascontiguousarray(out, dtype=np.float32)


def kernel(features, W1, b1, W2, b2, src, dst):
    return _gcn(features, W1, b1, W2, b2, src, dst)


# revision 17
# speedup vs baseline: 1.0441x; 1.0006x over previous
"""2-layer GCN (segment-mean -> Linear -> ReLU -> segment-mean -> Linear) on
8 Trainium2 NeuronCores.

Strategy:
- Shard dst nodes (and their incident edges) across the 8 cores.
- Edges sorted by (dst tile, src bucket); src features fetched with
  dma_gather (4 parallel SWDGE queues) as bf16 rows.
- Segment-sum per 128-dst tile via one-hot matmuls accumulated in PSUM;
  one-hot matrices built in a single DVE is_equal op per tile.
- Linear layers fused per tile (W1 + ReLU + W2); the [N, 64] intermediate
  z = relu(...) @ W2 is exchanged with 4 quarter-AllGathers issued as layer 1
  progresses, so the exchange overlaps the tail of layer 1's gathers and
  layer 2 starts without a full barrier.
- Uses the algebraic identity segmean(h) @ W = segmean(h @ W).
"""
import os
import numpy as np
import ml_dtypes

import concourse.bass as bass
import concourse.mybir as mybir
from concourse import bacc
from concourse.tile import TileContext
from concourse.masks import make_identity
from concourse.bass_utils import run_bass_kernel_spmd

P = 128
NCORES = 8
SRC_CHUNK = 25600           # int16-addressable table chunk (layer 1)
BF16 = ml_dtypes.bfloat16

LAST_EXEC_NS = None


def _bucketize(s_all, d_all, n_tiles, n_buckets, bucket_of, locidx_of, shard):
    """Build the uniform slot structure for one layer's gathers."""
    per_core = []
    counts = np.zeros((NCORES, n_tiles, n_buckets), np.int64)
    for c in range(NCORES):
        s, d = s_all[c], d_all[c]
        tile = d // P
        bucket = bucket_of(s)
        key = tile * n_buckets + bucket
        order = np.argsort(key, kind="stable")
        s, d, key = s[order], d[order], key[order]
        counts[c] = np.bincount(key, minlength=n_tiles * n_buckets).reshape(n_tiles, n_buckets)
        per_core.append((s, d, key))

    Lt_true = counts.max(axis=0)
    L_tb = ((Lt_true + P - 1) // P) * P
    empty = L_tb.sum(axis=1) == 0
    L_tb[empty, 0] = P
    Lt_true = np.maximum(Lt_true, (L_tb > 0).astype(np.int64))
    nch_tb = L_tb // P
    nch_t = nch_tb.sum(axis=1)
    total_chunks = int(nch_t.sum())
    total_slots = total_chunks * P
    chunk_off_tb = np.concatenate([[0], np.cumsum(nch_tb.reshape(-1))])[:-1].reshape(n_tiles, n_buckets)

    idx_list, dl_list = [], []
    for c in range(NCORES):
        s, d, key = per_core[c]
        idx16 = np.zeros(total_slots, np.int16)
        dloc = np.full(total_slots, 999.0, np.float32)
        cnt_flat = counts[c].reshape(-1)
        seg_start = np.concatenate([[0], np.cumsum(cnt_flat)])[:-1]
        slots = (chunk_off_tb.reshape(-1) * P)[key] + (np.arange(len(s)) - seg_start[key])
        idx16[slots] = locidx_of(s).astype(np.int16)
        dloc[slots] = (d % P).astype(np.float32)
        w = idx16.reshape(-1, 16).T
        idx_list.append(np.tile(w, (8, 1)))                 # [128, S/16]
        dl_list.append(dloc.reshape(total_chunks, P).T.astype(BF16))

    return dict(L_tb=L_tb, Lt_true=Lt_true, nch_t=nch_t, chunk_off_tb=chunk_off_tb,
                total_chunks=total_chunks, total_slots=total_slots,
                idx=idx_list, dl=dl_list)


def _prep(src, dst, n_nodes):
    shard = n_nodes // NCORES
    n_tiles = (shard + P - 1) // P
    n_buckets = (n_nodes + SRC_CHUNK - 1) // SRC_CHUNK
    Q = shard // 4                                 # quarter size (3125)

    src = np.asarray(src, np.int64)
    dst = np.asarray(dst, np.int64)
    s_all, d_all, cnt_nodes = [], [], []
    for c in range(NCORES):
        m = (dst >= c * shard) & (dst < (c + 1) * shard)
        s_all.append(src[m])
        d = dst[m] - c * shard
        d_all.append(d)
        cnt_nodes.append(np.bincount(d, minlength=shard).astype(np.float64))

    st1 = _bucketize(s_all, d_all, n_tiles, n_buckets,
                     lambda s: s // SRC_CHUNK,
                     lambda s: s - (s // SRC_CHUNK) * SRC_CHUNK, shard)
    st2 = st1  # layer 2 reuses the same bucketing (gather from z_full)

    data = []
    for c in range(NCORES):
        cnt_node = cnt_nodes[c]
        recip = (1.0 / np.maximum(cnt_node, 1.0)).astype(np.float32)
        cntp = np.maximum(cnt_node, 1.0).astype(np.float32)
        recip_pad = np.ones(n_tiles * P, np.float32)
        recip_pad[:shard] = recip
        cntp_pad = np.ones(n_tiles * P, np.float32)
        cntp_pad[:shard] = cntp
        rb = np.broadcast_to(recip_pad.reshape(n_tiles, 1, P), (n_tiles, P, P)).astype(BF16)
        data.append(dict(idx=st1["idx"][c], dl=st1["dl"][c],
                         idx2=st2["idx"][c], dl2=st2["dl"][c],
                         rb=np.ascontiguousarray(rb),
                         rc=recip_pad.reshape(n_tiles, P, 1).astype(np.float32),
                         cr=cntp_pad.reshape(n_tiles, 1, P).astype(BF16)))

    struct = dict(n_tiles=n_tiles, n_buckets=n_buckets, shard=shard, Q=Q,
                  st1=st1, st2=st2)
    return struct, data


def _build(st, n_nodes):
    n_tiles, n_buckets = st["n_tiles"], st["n_buckets"]
    shard, Q = st["shard"], st["Q"]
    st1, st2 = st["st1"], st["st2"]
    max_nch = int(max(st1["nch_t"].max(), st2["nch_t"].max()))
    f32, bf16, i16 = mybir.dt.float32, mybir.dt.bfloat16, mybir.dt.int16

    nc = bacc.Bacc("TRN2", target_bir_lowering=False, debug=False,
                   num_devices=NCORES, num_swdge_queues=4,
                   dynamic_dma_scratch_size=32768)
    X_d = nc.dram_tensor("X", [n_nodes, P], bf16, kind="ExternalInput")
    W1_d = nc.dram_tensor("W1b", [P, P], bf16, kind="ExternalInput")
    W2_d = nc.dram_tensor("W2b", [P, 64], bf16, kind="ExternalInput")
    b1_d = nc.dram_tensor("b1c", [P, 1], f32, kind="ExternalInput")
    b2_d = nc.dram_tensor("b2r", [1, 64], bf16, kind="ExternalInput")
    iota_d = nc.dram_tensor("iota", [P, P], bf16, kind="ExternalInput")
    idx_d = nc.dram_tensor("idx", [P, st1["total_slots"] // 16], i16, kind="ExternalInput")
    dl_d = nc.dram_tensor("dl", [P, st1["total_chunks"]], bf16, kind="ExternalInput")
    idx2_d = nc.dram_tensor("idx2", [P, st2["total_slots"] // 16], i16, kind="ExternalInput")
    dl2_d = nc.dram_tensor("dl2", [P, st2["total_chunks"]], bf16, kind="ExternalInput")
    rb_d = nc.dram_tensor("rb", [n_tiles, P, P], bf16, kind="ExternalInput")
    rc_d = nc.dram_tensor("rc", [n_tiles, P, 1], f32, kind="ExternalInput")
    cr_d = nc.dram_tensor("cr", [n_tiles, 1, P], bf16, kind="ExternalInput")
    out_d = nc.dram_tensor("out", [shard, 64], f32, kind="ExternalOutput")

    z_local = nc.dram_tensor("z_local", [shard, P], bf16)
    z_full = nc.dram_tensor("z_full", [NCORES * shard, P], bf16, addr_space="Shared")

    qn = [0]

    with TileContext(nc) as tc:
        with tc.tile_pool(name="const", bufs=1) as cpool, \
             tc.tile_pool(name="g", bufs=6) as gpool, \
             tc.tile_pool(name="oh", bufs=6) as ohpool, \
             tc.tile_pool(name="wk", bufs=3) as wpool, \
             tc.tile_pool(name="sm", bufs=3) as smpool, \
             tc.tile_pool(name="ps1", bufs=2, space="PSUM") as ps1, \
             tc.tile_pool(name="ps2", bufs=2, space="PSUM") as ps2, \
             tc.tile_pool(name="ps3", bufs=2, space="PSUM") as ps3, \
             tc.tile_pool(name="ps4", bufs=2, space="PSUM") as ps4:

            W1sb = cpool.tile([P, P], bf16)
            nc.sync.dma_start(out=W1sb[:], in_=W1_d[:])
            W2sb = cpool.tile([P, 64], bf16)
            nc.sync.dma_start(out=W2sb[:], in_=W2_d[:])
            b1sb = cpool.tile([P, 1], f32)
            nc.sync.dma_start(out=b1sb[:], in_=b1_d[:])
            b2sb = cpool.tile([1, 64], bf16)
            nc.sync.dma_start(out=b2sb[:], in_=b2_d[:])
            iotasb = cpool.tile([P, P], bf16)
            nc.sync.dma_start(out=iotasb[:], in_=iota_d[:])
            idxsb = cpool.tile([P, st1["total_slots"] // 16], i16)
            nc.sync.dma_start(out=idxsb[:], in_=idx_d[:])
            dlsb = cpool.tile([P, st1["total_chunks"]], bf16)
            nc.sync.dma_start(out=dlsb[:], in_=dl_d[:])
            idx2sb = cpool.tile([P, st2["total_slots"] // 16], i16)
            nc.sync.dma_start(out=idx2sb[:], in_=idx2_d[:])
            dl2sb = cpool.tile([P, st2["total_chunks"]], bf16)
            nc.sync.dma_start(out=dl2sb[:], in_=dl2_d[:])
            ident = cpool.tile([P, P], bf16)
            make_identity(nc, ident[:])

            for layer in (0, 1):
                s = st1 if layer == 0 else st2
                L_tb, Lt_true = s["L_tb"], s["Lt_true"]
                nch_t, chunk_off_tb = s["nch_t"], s["chunk_off_tb"]
                ixsb = idxsb if layer == 0 else idx2sb
                dsb = dlsb if layer == 0 else dl2sb
                nb = n_buckets
                for t in range(n_tiles):
                    nch = int(nch_t[t])
                    G = gpool.tile([P, max_nch * P], bf16, tag="G")
                    for b in range(nb):
                        L = int(L_tb[t, b])
                        if L == 0:
                            continue
                        if layer == 0:
                            table = X_d[b * SRC_CHUNK:min((b + 1) * SRC_CHUNK, n_nodes), :]
                        else:
                            table = z_full[b * SRC_CHUNK:min((b + 1) * SRC_CHUNK, n_nodes), :]
                        co = int(chunk_off_tb[t, b] - chunk_off_tb[t, 0])
                        gco = int(chunk_off_tb[t, b])
                        # first few tiles emit full padded counts so G pool
                        # buffers never expose uninitialized SBUF to the MMs
                        ntrue = L if (layer == 0 and t < 6) else int(Lt_true[t, b])
                        ncols = (ntrue + 15) // 16
                        gi = nc.gpsimd.dma_gather(
                            G[:, co * P:(co + L // P) * P].rearrange("p (c d) -> p c d", d=P),
                            table,
                            ixsb[:, gco * 8:gco * 8 + ncols],
                            ntrue, ntrue, P,
                            queue_num=qn[0] % 4,
                        )
                        qn[0] += 1
                    oh = ohpool.tile([P, max_nch * P], bf16, tag="oh")
                    dcol0 = int(chunk_off_tb[t, 0])
                    in0 = iotasb[:].rearrange("p (o d) -> p o d", o=1).broadcast_to([P, nch, P])
                    in1 = dsb[:, dcol0:dcol0 + nch].rearrange("p (c o) -> p c o", o=1).broadcast_to([P, nch, P])
                    nc.vector.tensor_tensor(
                        out=oh[:, :nch * P].rearrange("p (c d) -> p c d", d=P),
                        in0=in0, in1=in1, op=mybir.AluOpType.is_equal)
                    psum1 = ps1.tile([P, P], f32, space="PSUM", tag="p1")
                    for cci in range(nch):
                        nc.tensor.matmul(
                            out=psum1[:], lhsT=G[:, cci * P:(cci + 1) * P],
                            rhs=oh[:, cci * P:(cci + 1) * P],
                            start=(cci == 0), stop=(cci == nch - 1))
                    rows = min(P, shard - t * P)
                    if layer == 0:
                        rbt = smpool.tile([P, P], bf16, tag="rbt")
                        nc.sync.dma_start(out=rbt[:], in_=rb_d[t])
                        m1 = wpool.tile([P, P], bf16, tag="m1")
                        nc.vector.tensor_tensor(out=m1[:], in0=psum1[:], in1=rbt[:],
                                                op=mybir.AluOpType.mult)
                        psum2 = ps2.tile([P, P], f32, space="PSUM", tag="p2")
                        nc.tensor.matmul(out=psum2[:], lhsT=W1sb[:], rhs=m1[:],
                                         start=True, stop=True)
                        h1T = wpool.tile([P, P], bf16, tag="h1T")
                        nc.scalar.activation(out=h1T[:], in_=psum2[:],
                                             func=mybir.ActivationFunctionType.Relu,
                                             bias=b1sb[:, :1], scale=1.0)
                        psum3 = ps3.tile([64, P], f32, space="PSUM", tag="p3")
                        nc.tensor.matmul(out=psum3[:], lhsT=W2sb[:], rhs=h1T[:],
                                         start=True, stop=True)
                        zT = wpool.tile([64, P], bf16, tag="zT")
                        nc.scalar.activation(out=zT[:], in_=psum3[:],
                                             func=mybir.ActivationFunctionType.Copy,
                                             scale=1.0)
                        psum4 = ps4.tile([P, 64], f32, space="PSUM", tag="p4")
                        nc.tensor.matmul(out=psum4[:], lhsT=zT[:], rhs=ident[:64, :64],
                                         start=True, stop=True)
                        zt = wpool.tile([P, 64], bf16, tag="zt")
                        nc.scalar.activation(out=zt[:], in_=psum4[:],
                                             func=mybir.ActivationFunctionType.Copy,
                                             scale=1.0)
                        nc.sync.dma_start(out=z_local[t * P:t * P + rows, :64],
                                          in_=zt[:rows, :])
                    else:
                        s5 = wpool.tile([64, P], bf16, tag="zT")
                        nc.scalar.activation(out=s5[:], in_=psum1[:64, :],
                                             func=mybir.ActivationFunctionType.Copy,
                                             scale=1.0)
                        psum4b = ps4.tile([P, 64], f32, space="PSUM", tag="p4")
                        nc.tensor.matmul(out=psum4b[:], lhsT=s5[:], rhs=ident[:64, :64],
                                         start=True, stop=False)
                        crt = smpool.tile([1, P], bf16, tag="crt")
                        nc.sync.dma_start(out=crt[:], in_=cr_d[t])
                        nc.tensor.matmul(out=psum4b[:], lhsT=crt[:], rhs=b2sb[:],
                                         start=False, stop=True)
                        rct = smpool.tile([P, 1], f32, tag="rct")
                        nc.sync.dma_start(out=rct[:], in_=rc_d[t])
                        outt = wpool.tile([P, 64], f32, tag="outt")
                        nc.scalar.activation(out=outt[:], in_=psum4b[:],
                                             func=mybir.ActivationFunctionType.Copy,
                                             scale=rct[:, :1])
                        nc.sync.dma_start(out=out_d[t * P:t * P + rows, :],
                                          in_=outt[:rows, :])
                if layer == 0:
                    nc.gpsimd.collective_compute(
                        "AllGather", mybir.AluOpType.bypass,
                        replica_groups=[list(range(NCORES))],
                        ins=[z_local[:]], outs=[z_full[:]])
    nc.compile()
    return nc


def _gcn(features, W1, b1, W2, b2, src, dst):
    global LAST_EXEC_NS
    n_nodes = features.shape[0]
    st, data = _prep(src, dst, n_nodes)

    X16 = np.ascontiguousarray(np.asarray(features, np.float32)).astype(BF16)
    iota_host = np.tile(np.arange(P, dtype=np.float32)[None, :], (P, 1)).astype(BF16)
    common = dict(
        X=X16,
        W1b=np.asarray(W1, np.float32).astype(BF16),
        W2b=np.asarray(W2, np.float32).astype(BF16),
        b1c=np.asarray(b1, np.float32).reshape(P, 1),
        b2r=np.asarray(b2, np.float32).reshape(1, 64).astype(BF16),
        iota=iota_host,
    )
    in_maps = []
    for c in range(NCORES):
        d = data[c]
        in_maps.append(dict(common, idx=d["idx"], dl=d["dl"], idx2=d["idx2"],
                            dl2=d["dl2"], rb=d["rb"], rc=d["rc"], cr=d["cr"]))

    nc = _build(st, n_nodes)
    trace = bool(os.environ.get("GCN_TRACE"))
    try:
        res = run_bass_kernel_spmd(nc, in_maps, list(range(NCORES)), trace=trace,
                                   tmpdir=os.environ.get("GCN_TMPDIR"))
    except Exception:
        if not trace:
            raise
        res = run_bass_kernel_spmd(nc, in_maps, list(range(NCORES)))
    LAST_EXEC_NS = res.exec_time_ns
    out = np.concatenate([res.results[c]["out"] for c in range(NCORES)], axis=0)
    return np.ascontiguousarray(out, dtype=np.float32)


def kernel(features, W1, b1, W2, b2, src, dst):
    return _gcn(features, W1, b1, W2, b2, src, dst)
